# revision 16
# baseline (speedup 1.0000x reference)
"""DistMult decoder kernel for 8 Trainium2 NeuronCores.

Computes out = (input1 * weight[type_index]) @ input2.T + bias with
input1 [8192, 512], input2 [8192, 512] in fp32, out [8192, 8192].

Sharding: rows of input1 (and thus rows of the output) are split across
the 8 cores; input2 / weight / bias are replicated. No communication.

Per-core device program (M = 1024 rows):
  - lhsT  [MT, P, KT*128] = w_r-scaled shard of input1, packed on host
    into per-m-tile blocks (one contiguous 128 KB DMA per m-tile)
  - rhs   [512, 8192]  = input2 transposed + cast to fp16 on host
  - fp16 operands run the PE at 1 cycle/row with fp32 PSUM accumulation
  - GEMM over 16 n-groups of 512 cols x 8 m-tiles x 4 k matmuls;
    215.8 ns/matmul steady state (the N=512 fp16 streaming floor)
  - output stored as fp16 (16 MB/core instead of 32) and upcast on the
    host; total quantization error ~5e-4 vs the 2e-2 gate

Schedule rationale (from trace analysis): ~6 us fixed preamble; first
DMA data lands ~8.7 us; during the head window all 8 cores load
simultaneously so per-core aggregate is only ~180-300 GB/s. 512-col
n-groups keep the critical first-matmul set small (512 KB rhs + 128 KB
lhsT), spread round-robin over the three DGE rings in consumption
order. Warmup matmuls on zeroed SBUF keep the PE busy from ~7.6 us so
the HAM clock gate opens (2.4 GHz) before the real stream starts.
"""

import os

import numpy as np

import concourse.bacc as bacc
import concourse.mybir as mybir
from concourse.bass_utils import run_bass_kernel_spmd
from concourse.tile import TileContext

N_CORES = 8
N1, N2, D = 8192, 8192, 512
M = N1 // N_CORES  # rows per core
P = 128            # partitions
KT = D // P        # 4 k-tiles
MT = M // P        # 8 m-tiles
NG = 512           # n columns per group (one psum bank)
NT = N2 // NG      # 16 n-groups
NWARM = 12         # warmup matmuls: spans PE-ready (~7.6 us) to data-ready (~12.9)

TRACE = os.environ.get("BASS_KERNEL_TRACE", "0") == "1"
LAST_RESULTS = None

_cached_nc = None


def _build():
    nc = bacc.Bacc(
        "TRN2", target_bir_lowering=False, debug=False, enable_asserts=False, num_devices=N_CORES
    )
    f32 = mybir.dt.float32
    f16 = mybir.dt.float16
    lhsT = nc.dram_tensor("lhsT", [MT, P, KT * P], f16, kind="ExternalInput")
    rhs = nc.dram_tensor("rhs", [D, N2], f16, kind="ExternalInput")
    biasv = nc.dram_tensor("biasv", [P, 1], f32, kind="ExternalInput")
    out = nc.dram_tensor("out", [M, N2], f16, kind="ExternalOutput")

    # K-major DRAM view split into [P, KT, cols]: rhs_r[p, kt, n] is
    # rhs row kt*128+p, matching the per-k-tile partition layout.
    rhs_r = rhs[:, :].rearrange("(kt p) n -> p kt n", p=P)

    with TileContext(nc) as tc:
        with (
            tc.tile_pool(name="const", bufs=1) as constp,
            tc.tile_pool(name="lhs", bufs=1) as lhsp,
            tc.tile_pool(name="rhsp", bufs=4) as rhsp,
            tc.tile_pool(name="outp", bufs=8) as outp,
            tc.tile_pool(name="psum", bufs=4, space="PSUM") as psump,
        ):
            # Warmup tiles zeroed on GpSimd (ready first after preamble).
            warm_w = constp.tile([P, P], f16, tag="warmw")
            warm_r = constp.tile([P, NG], f16, tag="warmr")
            nc.gpsimd.memset(warm_w[:], 0.0)
            nc.gpsimd.memset(warm_r[:], 0.0)

            lt = lhsp.tile([P, MT, KT * P], f16, tag="lhs")
            bias_t = constp.tile([P, 1], f32, tag="bias")
            rts = {}

            def rtile(g):
                rt = rhsp.tile([P, KT, NG], f16, tag="rhs")
                rts[g] = rt
                return rt

            rt0, rt1 = rtile(0), rtile(1)

            # Priority-ordered head loads, round-robin across the three
            # DGE rings in consumption order (~128 KB pieces). The real
            # stream needs rt0 + lt m0 first; g1's quarters and the
            # later m-tiles interleave by their deadlines.
            nc.scalar.dma_start(out=bias_t[:], in_=biasv[:, :])
            nc.sync.dma_start(out=rt0[:, 0, :], in_=rhs_r[:, 0, 0:NG])
            nc.scalar.dma_start(out=lt[:, 0, :], in_=lhsT[0, :, :])
            nc.gpsimd.dma_start(out=rt0[:, 1, :], in_=rhs_r[:, 1, 0:NG])
            nc.sync.dma_start(out=lt[:, 1, :], in_=lhsT[1, :, :])
            nc.gpsimd.dma_start(out=rt0[:, 2, :], in_=rhs_r[:, 2, 0:NG])
            nc.scalar.dma_start(out=rt0[:, 3, :], in_=rhs_r[:, 3, 0:NG])
            nc.sync.dma_start(out=lt[:, 2, :], in_=lhsT[2, :, :])
            nc.scalar.dma_start(out=lt[:, 3, :], in_=lhsT[3, :, :])
            nc.gpsimd.dma_start(out=lt[:, 4, :], in_=lhsT[4, :, :])
            nc.sync.dma_start(out=rt1[:, 0, :], in_=rhs_r[:, 0, NG : 2 * NG])
            nc.scalar.dma_start(out=lt[:, 5, :], in_=lhsT[5, :, :])
            nc.gpsimd.dma_start(out=rt1[:, 1, :], in_=rhs_r[:, 1, NG : 2 * NG])
            nc.sync.dma_start(out=lt[:, 6, :], in_=lhsT[6, :, :])
            nc.scalar.dma_start(out=rt1[:, 2, :], in_=rhs_r[:, 2, NG : 2 * NG])
            nc.gpsimd.dma_start(out=lt[:, 7, :], in_=lhsT[7, :, :])
            nc.sync.dma_start(out=rt1[:, 3, :], in_=rhs_r[:, 3, NG : 2 * NG])

            # Warm up the PE's HAM clock gate during the head-load
            # window so the real matmuls start at 2.4 GHz.
            wps = psump.tile([P, NG], f32, tag="ps")
            for i in range(NWARM):
                nc.tensor.matmul(
                    wps[:], warm_w[:], warm_r[:],
                    start=(i == 0), stop=(i == NWARM - 1),
                )

            # Steady-state rhs prefetch on the GpSimd (SWDGE) queue:
            # latency-tolerant, never behind the HWDGE store streams.
            def load_rhs(g):
                rt = rtile(g)
                nc.gpsimd.dma_start(
                    out=rt[:], in_=rhs_r[:, :, g * NG : (g + 1) * NG]
                )

            for g in range(NT):
                rt = rts.pop(g)
                for m in range(MT):
                    if m == 0 and 2 <= g + 2 < NT:
                        load_rhs(g + 2)
                    last = g == NT - 1 and m == MT - 1
                    ps = psump.tile([P, NG], f32, tag="ps")
                    for k in range(KT):
                        nc.tensor.matmul(
                            ps[:], lt[:, m, k * P : (k + 1) * P],
                            rt[:, k, :],
                            start=(k == 0), stop=(k == KT - 1),
                        )
                    ot = outp.tile([P, NG], f16, tag="ot")
                    if last:
                        # Final tile: split the copy between ACT and DVE
                        # and the store over both HWDGE rings so the
                        # exit barrier waits on minimal serial work.
                        nc.scalar.activation(
                            ot[:, 0:256], ps[:, 0:256],
                            mybir.ActivationFunctionType.Identity,
                            bias=bias_t[:, 0:1],
                        )
                        nc.vector.tensor_scalar_add(
                            ot[:, 256:NG], ps[:, 256:NG], bias_t[:, 0:1]
                        )
                        nc.sync.dma_start(
                            out=out[m * P : (m + 1) * P,
                                    g * NG : g * NG + 256],
                            in_=ot[:, 0:256],
                        )
                        nc.scalar.dma_start(
                            out=out[m * P : (m + 1) * P,
                                    g * NG + 256 : (g + 1) * NG],
                            in_=ot[:, 256:NG],
                        )
                    else:
                        # Alternate psum->sbuf+bias copies between ACT
                        # and DVE, and stores between the HWDGE rings.
                        if m % 2 == 0:
                            nc.scalar.activation(
                                ot[:], ps[:],
                                mybir.ActivationFunctionType.Identity,
                                bias=bias_t[:, 0:1],
                            )
                        else:
                            nc.vector.tensor_scalar_add(
                                ot[:], ps[:], bias_t[:, 0:1]
                            )
                        st = nc.sync if m % 2 == 0 else nc.scalar
                        st.dma_start(
                            out=out[m * P : (m + 1) * P,
                                    g * NG : (g + 1) * NG],
                            in_=ot[:],
                        )
    nc.compile()
    return nc


def kernel(input1, input2, weight, bias, type_index):
    global _cached_nc, LAST_RESULTS

    input1 = np.asarray(input1, dtype=np.float32)
    input2 = np.asarray(input2, dtype=np.float32)
    weight = np.asarray(weight, dtype=np.float32)
    bias = np.asarray(bias, dtype=np.float32).reshape(-1)
    w_r = weight[int(type_index)]  # [D]

    # Host-side prep: fold the w_r row-scale into input1, lay both GEMM
    # operands out K-major, cast to fp16 (device accumulates in fp32).
    scaled = input1 * w_r[None, :]  # [N1, D]
    rhsT = np.ascontiguousarray(input2.T.astype(np.float16))  # [D, N2]
    bias_vec = np.full((P, 1), float(bias[0]), dtype=np.float32)

    in_maps = []
    for c in range(N_CORES):
        shard = scaled[c * M : (c + 1) * M]  # [M, D]
        # Pack per-m-tile weight blocks: lhsT[m, p, k*128+j] =
        # shard[m*128+j, k*128+p], so each m-tile is one contiguous DMA
        # and each k slice is a [K=128, M=128] stationary operand.
        a = shard.T.astype(np.float16).reshape(KT, P, MT, P)
        lhsT_packed = np.ascontiguousarray(
            a.transpose(2, 1, 0, 3).reshape(MT, P, KT * P)
        )
        in_maps.append(
            {
                "lhsT": lhsT_packed,
                "rhs": rhsT,
                "biasv": bias_vec,
            }
        )

    if _cached_nc is None:
        _cached_nc = _build()

    res = run_bass_kernel_spmd(
        _cached_nc, in_maps, core_ids=list(range(N_CORES)), trace=TRACE
    )
    LAST_RESULTS = res
    return np.concatenate(
        [res.results[c]["out"] for c in range(N_CORES)], axis=0
    ).astype(np.float32)


# revision 19
# speedup vs baseline: 1.1799x; 1.1799x over previous
"""DistMult decoder kernel for 8 Trainium2 NeuronCores.

Computes out = (input1 * weight[type_index]) @ input2.T + bias with
input1 [8192, 512], input2 [8192, 512] in fp32, out [8192, 8192].

Sharding: rows of input1 (and thus rows of the output) are split across
the 8 cores; input2 / weight / bias are replicated. No communication.

Split-K mixed precision: the contraction is permutation-invariant, so
the host sorts the 512 k-coordinates by |w_r| and sends the 256
SMALLEST-|w| coordinates through fp8e4 + DoubleRow (one 256-row DR
matmul at ~2 rows/cycle replaces two fp16 matmuls) and the 256 largest
through fp16. Both error terms scale with sum(w_r^2) over the fp8 set,
which carries only ~8% of the weight mass: measured max-rel error
1.14e-2 (gate 2e-2) vs 4.7e-4 all-fp16 and 3.5e-2 for an unsorted
split. PE work per output tile drops from 4 to ~3 matmul slots.

Per-core device program (M = 1024 rows):
  - lhsT16 [MT, P, 256] fp16 / lhsT8 [MT, P, 2*128] fp8e4: per-m-tile
    packed stationary operands (fp8 pair = two stacked 128-row blocks,
    the plain-DoubleRow [Ki, 2, M] layout)
  - rhs16 [256, 8192] fp16 / rhs8 [256, 8192] fp8e4, K-major
  - 16 n-groups of 512 cols x 8 m-tiles x (2 fp16 + 1 DR) matmuls into
    one PSUM bank; fp32 accumulate; fp16 output stores, host upcast
  - head DMAs round-robin the three DGE rings in consumption order;
    12 warmup matmuls span PE-ready (~7.6us) to data-ready so the HAM
    clock gate opens before the real stream starts
"""

import os

import numpy as np
import ml_dtypes

import concourse.bacc as bacc
import concourse.mybir as mybir
from concourse.bass_utils import run_bass_kernel_spmd
from concourse.tile import TileContext

N_CORES = 8
N1, N2, D = 8192, 8192, 512
M = N1 // N_CORES  # rows per core
P = 128            # partitions
MT = M // P        # 8 m-tiles
NG = 512           # n columns per group (one psum bank)
NT = N2 // NG      # 16 n-groups
KH = 256           # k-coordinates per precision half
NWARM = 12         # warmup matmuls: spans PE-ready (~7.6 us) to data-ready (~12.9)

TRACE = os.environ.get("BASS_KERNEL_TRACE", "0") == "1"
LAST_RESULTS = None

_cached_nc = None


def _build():
    nc = bacc.Bacc(
        "TRN2", target_bir_lowering=False, debug=False, enable_asserts=False, num_devices=N_CORES
    )
    f32 = mybir.dt.float32
    f16 = mybir.dt.float16
    f8 = mybir.dt.float8e4
    lhsT16 = nc.dram_tensor("lhsT16", [MT, P, KH], f16, kind="ExternalInput")
    lhsT8 = nc.dram_tensor("lhsT8", [MT, P, KH], f8, kind="ExternalInput")
    rhs16 = nc.dram_tensor("rhs16", [KH, N2], f16, kind="ExternalInput")
    rhs8 = nc.dram_tensor("rhs8", [KH, N2], f8, kind="ExternalInput")
    biasv = nc.dram_tensor("biasv", [P, 1], f32, kind="ExternalInput")
    out = nc.dram_tensor("out", [M, N2], f16, kind="ExternalOutput")

    # K-major views split into [P, 2, cols]: row kt*128+p.
    rhs16_r = rhs16[:, :].rearrange("(kt p) n -> p kt n", p=P)
    rhs8_r = rhs8[:, :].rearrange("(kt p) n -> p kt n", p=P)

    with TileContext(nc) as tc:
        with (
            tc.tile_pool(name="const", bufs=1) as constp,
            tc.tile_pool(name="lhs", bufs=1) as lhsp,
            tc.tile_pool(name="r16p", bufs=4) as r16p,
            tc.tile_pool(name="r8p", bufs=4) as r8p,
            tc.tile_pool(name="outp", bufs=8) as outp,
            tc.tile_pool(name="psum", bufs=4, space="PSUM") as psump,
        ):
            # Warmup tiles zeroed on GpSimd (ready first after preamble).
            warm_w = constp.tile([P, P], f16, tag="warmw")
            warm_r = constp.tile([P, NG], f16, tag="warmr")
            nc.gpsimd.memset(warm_w[:], 0.0)
            nc.gpsimd.memset(warm_r[:], 0.0)

            lt16 = lhsp.tile([P, MT, KH], f16, tag="l16")
            lt8 = lhsp.tile([P, MT, 2, P], f8, tag="l8")
            bias_t = constp.tile([P, 1], f32, tag="bias")
            r16s, r8s = {}, {}

            def rtiles(g):
                r16 = r16p.tile([P, 2, NG], f16, tag="r16")
                r8 = r8p.tile([P, 2, NG], f8, tag="r8")
                r16s[g] = r16
                r8s[g] = r8
                return r16, r8

            ra16, ra8 = rtiles(0)
            rb16, rb8 = rtiles(1)

            # Priority-ordered head loads, round-robin across the three
            # DGE rings in consumption order (~64-128 KB pieces).
            nc.sync.dma_start(out=ra16[:, 0, :], in_=rhs16_r[:, 0, 0:NG])
            nc.scalar.dma_start(out=bias_t[:], in_=biasv[:, :])
            nc.scalar.dma_start(out=lt16[:, 0, :], in_=lhsT16[0, :, :])
            nc.gpsimd.dma_start(out=ra16[:, 1, :], in_=rhs16_r[:, 1, 0:NG])
            nc.sync.dma_start(out=ra8[:], in_=rhs8_r[:, :, 0:NG])
            nc.scalar.dma_start(
                out=lt8[:, 0:2], in_=lhsT8[0:2, :, :].rearrange("m p j -> p m j")
            )
            nc.gpsimd.dma_start(out=lt16[:, 1, :], in_=lhsT16[1, :, :])
            nc.sync.dma_start(out=lt16[:, 2, :], in_=lhsT16[2, :, :])
            nc.scalar.dma_start(out=lt16[:, 3, :], in_=lhsT16[3, :, :])
            nc.gpsimd.dma_start(
                out=lt8[:, 2:4], in_=lhsT8[2:4, :, :].rearrange("m p j -> p m j")
            )
            nc.sync.dma_start(out=lt16[:, 4, :], in_=lhsT16[4, :, :])
            nc.scalar.dma_start(
                out=lt8[:, 4:6], in_=lhsT8[4:6, :, :].rearrange("m p j -> p m j")
            )
            nc.gpsimd.dma_start(out=lt16[:, 5, :], in_=lhsT16[5, :, :])
            nc.sync.dma_start(out=lt16[:, 6, :], in_=lhsT16[6, :, :])
            nc.scalar.dma_start(
                out=lt8[:, 6:8], in_=lhsT8[6:8, :, :].rearrange("m p j -> p m j")
            )
            nc.gpsimd.dma_start(out=lt16[:, 7, :], in_=lhsT16[7, :, :])
            nc.sync.dma_start(out=rb16[:], in_=rhs16_r[:, :, NG : 2 * NG])
            nc.scalar.dma_start(out=rb8[:], in_=rhs8_r[:, :, NG : 2 * NG])

            # Warm up the PE's HAM clock gate during the head-load
            # window so the real matmuls start at 2.4 GHz.
            wps = psump.tile([P, NG], f32, tag="ps")
            for i in range(NWARM):
                nc.tensor.matmul(
                    wps[:], warm_w[:], warm_r[:],
                    start=(i == 0), stop=(i == NWARM - 1),
                )

            # Steady-state rhs prefetch on the GpSimd (SWDGE) queue.
            def load_rhs(g):
                r16, r8 = rtiles(g)
                nc.gpsimd.dma_start(
                    out=r16[:], in_=rhs16_r[:, :, g * NG : (g + 1) * NG]
                )
                nc.gpsimd.dma_start(
                    out=r8[:], in_=rhs8_r[:, :, g * NG : (g + 1) * NG]
                )

            for g in range(NT):
                r16 = r16s.pop(g)
                r8 = r8s.pop(g)
                for m in range(MT):
                    if m == 0 and 2 <= g + 2 < NT:
                        load_rhs(g + 2)
                    last = g == NT - 1 and m == MT - 1
                    ps = psump.tile([P, NG], f32, tag="ps")
                    nc.tensor.matmul(
                        ps[:], lt16[:, m, 0:P], r16[:, 0, :],
                        start=True, stop=False,
                    )
                    nc.tensor.matmul(
                        ps[:], lt16[:, m, P:KH], r16[:, 1, :],
                        start=False, stop=False,
                    )
                    # DoubleRow: [Ki=128, 2, *] APs; computes
                    # sum_i lt8[:, m, i].T @ r8[:, i, :] at 2 rows/cycle.
                    nc.tensor.matmul(
                        ps[:], lt8[:, m], r8[:],
                        start=False, stop=True,
                        perf_mode=mybir.MatmulPerfMode.DoubleRow,
                    )
                    ot = outp.tile([P, NG], f16, tag="ot")
                    if last:
                        # Final tile: split the copy between ACT and DVE
                        # and the store over both HWDGE rings so the
                        # exit barrier waits on minimal serial work.
                        nc.scalar.activation(
                            ot[:, 0:256], ps[:, 0:256],
                            mybir.ActivationFunctionType.Identity,
                            bias=bias_t[:, 0:1],
                        )
                        nc.vector.tensor_scalar_add(
                            ot[:, 256:NG], ps[:, 256:NG], bias_t[:, 0:1]
                        )
                        nc.sync.dma_start(
                            out=out[m * P : (m + 1) * P,
                                    g * NG : g * NG + 256],
                            in_=ot[:, 0:256],
                        )
                        nc.scalar.dma_start(
                            out=out[m * P : (m + 1) * P,
                                    g * NG + 256 : (g + 1) * NG],
                            in_=ot[:, 256:NG],
                        )
                    else:
                        if m % 2 == 0:
                            nc.scalar.activation(
                                ot[:], ps[:],
                                mybir.ActivationFunctionType.Identity,
                                bias=bias_t[:, 0:1],
                            )
                        else:
                            nc.vector.tensor_scalar_add(
                                ot[:], ps[:], bias_t[:, 0:1]
                            )
                        st = nc.sync if m % 2 == 0 else nc.scalar
                        st.dma_start(
                            out=out[m * P : (m + 1) * P,
                                    g * NG : (g + 1) * NG],
                            in_=ot[:],
                        )
    nc.compile()
    return nc


def _pack_lhs(shard):
    """[M, KH] -> [MT, P, KH] with [m, p, kt*128+j] = shard[m*128+j, kt*128+p]."""
    a = shard.T.reshape(2, P, MT, P)
    return np.ascontiguousarray(a.transpose(2, 1, 0, 3).reshape(MT, P, KH))


def kernel(input1, input2, weight, bias, type_index):
    global _cached_nc, LAST_RESULTS

    input1 = np.asarray(input1, dtype=np.float32)
    input2 = np.asarray(input2, dtype=np.float32)
    weight = np.asarray(weight, dtype=np.float32)
    bias = np.asarray(bias, dtype=np.float32).reshape(-1)
    w_r = weight[int(type_index)]  # [D]

    # Fold the w_r row-scale into input1, then split the contraction:
    # the 256 smallest-|w_r| coordinates go to fp8, the rest to fp16.
    order = np.argsort(np.abs(w_r))
    s8, s16 = order[:KH], order[KH:]
    scaled = input1 * w_r[None, :]  # [N1, D]
    a16_all = scaled[:, s16]
    a8_all = scaled[:, s8]
    f8 = ml_dtypes.float8_e4m3
    rhsT16 = np.ascontiguousarray(input2[:, s16].T.astype(np.float16))
    rhsT8 = np.ascontiguousarray(input2[:, s8].T.astype(f8))
    bias_vec = np.full((P, 1), float(bias[0]), dtype=np.float32)

    in_maps = []
    for c in range(N_CORES):
        in_maps.append(
            {
                "lhsT16": _pack_lhs(
                    a16_all[c * M : (c + 1) * M].astype(np.float16)
                ),
                "lhsT8": _pack_lhs(a8_all[c * M : (c + 1) * M].astype(f8)),
                "rhs16": rhsT16,
                "rhs8": rhsT8,
                "biasv": bias_vec,
            }
        )

    if _cached_nc is None:
        _cached_nc = _build()

    res = run_bass_kernel_spmd(
        _cached_nc, in_maps, core_ids=list(range(N_CORES)), trace=TRACE
    )
    LAST_RESULTS = res
    return np.concatenate(
        [res.results[c]["out"] for c in range(N_CORES)], axis=0
    ).astype(np.float32)
